# revision 49
# baseline (speedup 1.0000x reference)
"""ClsbdCRF message-passing kernel for 8 Trainium2 NeuronCores.

Sharding: core i handles batch b = i//2 and image-row half i%2 (64 output
rows each, halos sliced host-side).  Per-core SBUF layout puts W=128 on
partitions and (C, H) on the free dimension.

Formulation ("input frame"): for every tap t the reference computes
  msg[c,p] += w_t[p] * xp[c, p+t].
We instead build the tap weight shifted into the *input* frame,
  w'_t[u] = w_t[u - t],
multiply v_t = w'_t * xp (per-tap elementwise product, fp16, DVE 2x mode),
and let the PE do the shift-and-accumulate into PSUM with 0/1 shift-matrix
stationaries:  msg[w] += v_t[w + dy_t]  (dx_t handled as a free-dim offset
baked into the exact 64-wide window of each product tile).

Products for taps sharing dx share the same xp window, so they are fused
into one stacked [W, K, C, 64] DVE multiply (weight slots grouped by dx);
the PE accumulation matmuls read per-tap slices of the group tile.

Weight structure exploited:
  * pairwise gaussian symmetry: g1'_t = g1_{-t}; 12 direct tiles cover 12
    taps for free, the other 10 (dy!=0) are small PE partition shifts.
  * clsbd ring-1: g2'_t[u] = s[u] for all 8 taps -> the pos-stream ring-1
    products depend only on dx -> 3 shared products, 8 matmuls.
  * ring-2: g2'_t[u] = max(s[u], s[u+d1], s[u+d2]) with |d|<=1.
  * center tap: msg_neg += c0 * xp via a scaled-identity stationary.
All products/weights fp16 (rel err ~5e-4, tolerance 2e-2); PSUM fp32.
"""

import math

import numpy as np

B, C, H, W, D = 4, 21, 128, 128, 5
EPS = 1e-5
HP = 64          # output rows per core
HE = HP + 4      # x row extent (halo 2)        frame: q in [0,68), idx = q
FE = HP + 8      # feats row extent (halo 4)    idx = q + 2
SE = HP + 6      # clsbd row extent (halo 3)    idx = q + 1
G1W = 66         # direct-gaussian window: q in [0,66)
BIGPAD = 1000.0
C0N = 2.0 - math.log(EPS)   # center neg weight (final *5 at evac)

RING1 = [(-1, -1), (-1, 0), (-1, 1), (0, -1), (0, 1), (1, -1), (1, 0), (1, 1)]
RING2 = [(-2, -2), (-2, -1), (-2, 0), (-2, 1), (-2, 2), (-1, -2), (-1, 2),
         (0, -2), (0, 2), (1, -2), (1, 2), (2, -2), (2, -1), (2, 0), (2, 1),
         (2, 2)]
EXP1 = [0, 0, 1, 2, 2, 0, 2, 3, 4, 5, 7, 5, 5, 6, 7, 7]
EXP2 = [0, 1, 1, 1, 2, 3, 4, 3, 4, 3, 4, 5, 6, 6, 6, 7]
ALLT = [(dx, dy) for dx in range(-2, 3) for dy in range(-2, 3)
        if (dx, dy) != (0, 0)]
DIRTAPS = [t for t in ALLT if t > (0, 0)]           # direct-gaussian taps
DYS5 = [-2, -1, 0, 1, 2]
JDY = {dy: j for j, dy in enumerate(DYS5)}           # fs / sm slot per dy

# direct-gaussian stack, dx-major (dy ascending inside) so the batched neg
# weight subs read uniform stride-1 slot runs
DSTACK = sorted(DIRTAPS, key=lambda t: (t[0], t[1]))
DSLOT = {t: i for i, t in enumerate(DSTACK)}

# canonical ring-2 order: dx-major, dy DESCENDING (matches neg slot order)
R2ORD = [t for dx in DYS5
         for t in sorted((u for u in RING2 if u[0] == dx),
                         key=lambda u: -u[1])]
R2IDX = {t: i for i, t in enumerate(R2ORD)}

# weight-stack slots (fp16, exact 64-wide windows)
PR1SLOT = {dx: dx + 1 for dx in (-1, 0, 1)}          # 0..2 pos ring-1 (by dx)
PR2SLOT = {t: 3 + R2IDX[t] for t in RING2}           # 3..18 pos ring-2
# neg slot groups (dy descending): [dir dx=-2 x5][dir dx=-1 x5][dir dx=0 x2]
#                                  [mir dx=0 x2][mix dx=1 x5][mix dx=2 x5]
NEGORD = ([(-2, dy) for dy in (2, 1, 0, -1, -2)]
          + [(-1, dy) for dy in (2, 1, 0, -1, -2)]
          + [(0, -1), (0, -2)]
          + [(0, 2), (0, 1)]
          + [(1, dy) for dy in (2, 1, 0, -1, -2)]
          + [(2, dy) for dy in (2, 1, 0, -1, -2)])
NEGSLOT = {t: 19 + i for i, t in enumerate(NEGORD)}
NW = 43

# mirror psum column layout (order = wstk mix-group order, fs excluded)
MIRTAPS = [(0, 2), (0, 1), (1, 2), (1, 1), (1, -1), (1, -2),
           (2, 2), (2, 1), (2, -1), (2, -2)]
MIRCOL = {t: i for i, t in enumerate(MIRTAPS)}

FSL = [(0, 512), (512, 1024), (1024, C * HP)]        # psum bank chunks
NMM = {"n": 25, "p": 24}                             # matmuls per chunk

_cache = {}


def _build():
    import concourse.bacc as bacc
    import concourse.mybir as mybir
    from concourse.tile import TileContext

    f32 = mybir.dt.float32
    f16 = mybir.dt.float16
    Act = mybir.ActivationFunctionType
    Alu = mybir.AluOpType

    nc = bacc.Bacc()
    xs_d = nc.declare_dram_parameter("xss", [W, C * HE + 6 * SE], f16,
                                     isOutput=False)
    f_d = nc.declare_dram_parameter("fs", [W, 2, 5, D, FE], f16,
                                    isOutput=False)
    m_d = nc.declare_dram_parameter("sm", [W, 6, W], f16, isOutput=False)
    o_d = nc.declare_dram_parameter("out", [2, W, C, HP], f32, isOutput=True)

    with TileContext(nc) as tc:
        with (
            tc.tile_pool(name="io", bufs=1) as io,
            tc.tile_pool(name="wk", bufs=1) as wk,
            tc.tile_pool(name="sc", bufs=1) as scp,
            tc.tile_pool(name="vp", bufs=3) as vp,
            tc.tile_pool(name="ps", bufs=1, space="PSUM") as psp,
        ):
            # ---- loads (all plain contiguous DMAs, spread over queues) ----
            xss_t = io.tile([W, C * HE + 6 * SE], f16, tag="xss")
            fs_t = io.tile([W, 2, 5, D, FE], f16, tag="fs")
            sm_t = io.tile([W, 6, W], f16, tag="sm")
            bc_t = io.tile([W, 3], f32, tag="bc")
            # biases via memsets: a [W,3] DMA costs a full 128-descriptor
            # chain (~6us) and would gate lnx
            nc.gpsimd.memset(bc_t[:, 0:1], EPS)
            nc.gpsimd.memset(bc_t[:, 1:2], math.log(2.0))
            nc.gpsimd.memset(bc_t[:, 2:3], 1.0 + EPS)
            # prime the Ln activation-table set during the DMA wait so lnx
            # does not pay the ~1.3us ACT_TABLE_LOAD on the critical chain
            warm = wk.tile([W, 1], f32, tag="warm")
            with tc.high_priority():
                nc.scalar.activation(warm[:], bc_t[:, 0:1], Act.Ln,
                                     bias=bc_t[:, 0:1], scale=1.0)
            HW2 = W // 2
            nc.sync.dma_start(out=xss_t[:HW2], in_=xs_d[:HW2])
            nc.scalar.dma_start(out=xss_t[HW2:], in_=xs_d[HW2:])
            nc.gpsimd.dma_start(out=fs_t[:], in_=f_d[:])
            nc.sync.dma_start(out=sm_t[:], in_=m_d[:])
            x_t = xss_t[:, 0:C * HE].rearrange("p (c h) -> p c h", c=C)
            ss_t = xss_t[:, C * HE:].bitcast(f32).rearrange(
                "p (s h) -> p s h", s=3)
            b_eps = bc_t[:, 0:1]
            b_ln2 = bc_t[:, 1:2]
            b_1eps = bc_t[:, 2:3]

            accn = psp.tile([W, 1536], f32, tag="accn")
            accp = psp.tile([W, 1536], f32, tag="accp")
            mir = psp.tile([W, 1024], f32, tag="mir")

            s0 = ss_t[:, 1]                       # s[u] at idx u+1

            # ---- ring-2 clsbd gaussians (R2ORD slots, runs batched) ----
            g2r2 = wk.tile([W, 16, HP], f32, tag="g2r2")
            deltas = []
            for t in R2ORD:
                dx, dy = t
                k = RING2.index(t)
                par = sorted({EXP1[k], EXP2[k]})
                deltas.append((2 + dx,
                               [(RING1[e][0] - dx, RING1[e][1] - dy)
                                for e in par]))
            sl = 0
            while sl < 16:          # pass 1: runs sharing (u0, delta1)
                u0, ds = deltas[sl]
                n = 1
                while sl + n < 16 and deltas[sl + n][0] == u0 \
                        and deltas[sl + n][1][0] == ds[0]:
                    n += 1
                dxx, dyy = ds[0]
                nc.vector.tensor_max(
                    g2r2[:, sl:sl + n],
                    s0[:, None, u0 + 1:u0 + 65].broadcast_to((W, n, HP)),
                    ss_t[:, 1 + dyy, None,
                         u0 + dxx + 1:u0 + dxx + 65].broadcast_to(
                             (W, n, HP)))
                sl += n
            for sl, (u0, ds) in enumerate(deltas):   # pass 2: 2nd parent
                if len(ds) > 1:
                    dxx, dyy = ds[1]
                    nc.vector.tensor_max(
                        g2r2[:, sl], g2r2[:, sl],
                        ss_t[:, 1 + dyy, u0 + dxx + 1:u0 + dxx + 65])

            # ---- polarness -> xp (fp16); high priority so the scheduler
            # prefers this chain over the fs-gated g1 subs (fills the DVE
            # bubble while the fs DMA is still landing) ----
            with tc.high_priority():
                lnx = scp.tile([W, C, HE], f16, tag="lnx")
                nc.scalar.activation(lnx[:], x_t, Act.Ln, bias=b_eps,
                                     scale=1.0)
                xl = scp.tile([W, C, HE], f16, tag="xl")
                nc.vector.tensor_mul(xl[:], x_t, lnx[:])
                e10 = scp.tile([W, 10, HE], f16, tag="e10")
                nc.vector.tensor_add(e10[:], xl[:, 0:10], xl[:, 10:20])
                e5 = scp.tile([W, 5, HE], f16, tag="e5")
                nc.vector.tensor_add(e5[:], e10[:, 0:5], e10[:, 5:10])
                e2 = scp.tile([W, 2, HE], f16, tag="e2")
                nc.vector.tensor_add(e2[:], e5[:, 0:2], e5[:, 2:4])
                ent = wk.tile([W, HE], f16, tag="ent")
                nc.vector.tensor_add(ent[:], e2[:, 0], e2[:, 1])
                nc.vector.tensor_add(ent[:], ent[:], e5[:, 4])
                nc.vector.tensor_add(ent[:], ent[:], xl[:, 20])
                pl = wk.tile([W, HE], f16, tag="pl")
                nc.scalar.activation(pl[:], ent[:], Act.Copy,
                                     bias=1.0, scale=1.0 / math.log(C))
                xp16e = io.tile([W, C, HE], f16, tag="xp16e")
                nc.vector.tensor_mul(
                    xp16e[:], x_t,
                    pl[:, None, :].broadcast_to((W, C, HE)))
                xp16o = io.tile([W, C, G1W], f16, tag="xp16o")
                nc.vector.tensor_copy(xp16o[:], xp16e[:, :, 1:1 + G1W])
            xpc = io.tile([W, C, HP], f16, tag="xpc")
            nc.vector.tensor_copy(xpc[:], xp16e[:, :, 2:2 + HP])

            def xp_win(dx):
                o = 2 + dx
                if o % 2 == 0:
                    return xp16e[:, :, o:o + HP]
                return xp16o[:, :, o - 1:o - 1 + HP]

            # ---- early scalar weights from s only ----
            wstk = wk.tile([W, NW, HP], f16, tag="wstk")
            lnn = wk.tile([W, HE], f16, tag="lnn")
            nc.scalar.activation(lnn[:], s0[:, 1:1 + HE], Act.Ln,
                                 bias=b_eps, scale=1.0)
            for dx in (-1, 0, 1):
                u0 = 2 + dx
                nc.scalar.activation(wstk[:, PR1SLOT[dx]],
                                     s0[:, u0 + 1:u0 + 65], Act.Ln,
                                     bias=b_1eps, scale=-1.0)

            # ---- matmul emission helpers (psum accumulation groups) ----
            cnt = {("n", i): 0 for i in range(3)} | {("p", i): 0
                                                     for i in range(3)}

            def emit_mm(stream, vflat, dy):
                ps = accn if stream == "n" else accp
                for ci, (n0, n1) in enumerate(FSL):
                    cnt[(stream, ci)] += 1
                    c = cnt[(stream, ci)]
                    nc.tensor.matmul(ps[:, n0:n1], sm_t[:, JDY[dy]],
                                     vflat[:, n0:n1], start=(c == 1),
                                     stop=(c == NMM[stream]),
                                     skip_group_check=True)

            def product(stream, wslot, dx, dys):
                # single-tap product, possibly accumulated with several dys
                v = vp.tile([W, C, HP], f16, tag="v")
                nc.vector.tensor_mul(
                    v[:], wstk[:, wslot, None, :].broadcast_to((W, C, HP)),
                    xp_win(dx))
                vflat = v[:].rearrange("p c h -> p (c h)")
                for dy in dys:
                    emit_mm(stream, vflat, dy)

            def gproduct(stream, slot0, taps):
                # fused product for K same-dx taps: [W, K, C, HP]
                dx = taps[0][0]
                K = len(taps)
                vg = vp.tile([W, K, C, HP], f16, tag=f"vg{K}")
                nc.vector.tensor_mul(
                    vg[:],
                    wstk[:, slot0:slot0 + K, None, :].broadcast_to(
                        (W, K, C, HP)),
                    xp_win(dx)[:, None, :, :].broadcast_to((W, K, C, HP)))
                for k, (tdx, tdy) in enumerate(taps):
                    emit_mm(stream, vg[:, k].rearrange("p c h -> p (c h)"),
                            tdy)

            # center tap first: warms PE early, opens the accn groups
            for ci, (n0, n1) in enumerate(FSL):
                cnt[("n", ci)] += 1
                nc.tensor.matmul(accn[:, n0:n1], sm_t[:, 5],
                                 xpc[:].rearrange("p c h -> p (c h)")[:, n0:n1],
                                 start=True, stop=(cnt[("n", ci)] == NMM["n"]),
                                 skip_group_check=True)

            # ---- pairwise gaussian chain ----
            dstk = scp.tile([W, 12, D, G1W], f16, tag="dstk")
            for t in DIRTAPS:
                mdx, mdy = t
                o = 2 + mdx
                if o % 2 == 0:
                    in1 = fs_t[:, 0, JDY[mdy], :, o:o + G1W]
                else:
                    in1 = fs_t[:, 1, JDY[mdy], :, o - 1:o - 1 + G1W]
                nc.vector.tensor_sub(
                    dstk[:, DSLOT[t]], fs_t[:, 0, 2, :, 2:2 + G1W], in1)
            sq = scp.tile([W, 12, D, G1W], f16, tag="sq")
            for a, b2 in ((0, 4), (4, 8), (8, 12)):
                nc.scalar.activation(sq[:, a:b2], dstk[:, a:b2],
                                     Act.Square, bias=0.0, scale=1.0)

            # pos ring-1: 3 shared products (weight depends only on dx),
            # each accumulated with 2-3 different dy shift matrices
            for dx in (-1, 0, 1):
                dys = [dy for dy in (-1, 0, 1) if (dx, dy) != (0, 0)]
                product("p", PR1SLOT[dx], dx, dys)

            # ---- sum over D (fp16 adds, 2x mode) ----
            q01 = scp.tile([W, 12, G1W], f16, tag="q01")
            nc.vector.tensor_add(q01[:], sq[:, :, 0], sq[:, :, 1])
            q23 = scp.tile([W, 12, G1W], f16, tag="q23")
            nc.vector.tensor_add(q23[:], sq[:, :, 2], sq[:, :, 3])
            nc.vector.tensor_add(q01[:], q01[:], q23[:])
            ssum = scp.tile([W, 12, G1W], f16, tag="ssum")
            nc.vector.tensor_add(ssum[:], q01[:], sq[:, :, 4])

            # ---- ring-2 weights (scalar; only need g2r2) + exp ----
            lnn2 = wk.tile([W, 16, HP], f16, tag="lnn2")
            nc.scalar.activation(lnn2[:], g2r2[:], Act.Ln,
                                 bias=b_eps, scale=1.0)
            nc.scalar.activation(wstk[:, 3:19], g2r2[:], Act.Ln,
                                 bias=b_1eps, scale=-1.0)
            g1x2 = wk.tile([W, 12, G1W], f16, tag="g1x2")
            nc.scalar.activation(g1x2[:], ssum[:], Act.Exp,
                                 bias=b_ln2, scale=-0.5)
            for dx in DYS5:
                taps = [t for t in R2ORD if t[0] == dx]
                gproduct("p", PR2SLOT[taps[0]], taps)

            def lnn_in1(t):
                if t in RING2:
                    return lnn2[:, R2IDX[t]]
                return lnn[:, 2 + t[0]:2 + t[0] + HP]

            # ---- neg direct weights: batched subs with uniform strides ----
            # dx=-2 group: all ring-2; direct slots (2,-dy) = 7..11 asc
            nc.vector.tensor_sub(wstk[:, 19:24], g1x2[:, 7:12, 0:64],
                                 lnn2[:, 0:5])
            gproduct("n", 19, [(-2, dy) for dy in (2, 1, 0, -1, -2)])
            # dx=-1 group: ring-1 middle run + 2 ring-2 singletons
            nc.vector.tensor_sub(
                wstk[:, 25:28], g1x2[:, 3:6, 1:65],
                lnn[:, None, 1:65].broadcast_to((W, 3, HP)))
            nc.vector.tensor_sub(wstk[:, 24], g1x2[:, 2, 1:65],
                                 lnn2[:, R2IDX[(-1, 2)]])
            nc.vector.tensor_sub(wstk[:, 28], g1x2[:, 6, 1:65],
                                 lnn2[:, R2IDX[(-1, -2)]])
            gproduct("n", 24, [(-1, dy) for dy in (2, 1, 0, -1, -2)])
            # dx=0 direct: (0,-1) ring1, (0,-2) ring2
            nc.vector.tensor_sub(wstk[:, 29], g1x2[:, 0, 2:66],
                                 lnn[:, 2:66])
            nc.vector.tensor_sub(wstk[:, 30], g1x2[:, 1, 2:66],
                                 lnn2[:, R2IDX[(0, -2)]])
            gproduct("n", 29, [(0, -1), (0, -2)])

            # ---- mirror taps via per-tap PE partition shifts ----
            g1den = wk.tile([W, 12, HP], f16, tag="g1den")
            nc.scalar.activation(g1den[:], g1x2[:, :, 2:2 + HP],
                                 Act.Copy, bias=0.0, scale=1.0)
            for t in MIRTAPS:
                dx, dy = t
                col = MIRCOL[t]
                nc.tensor.matmul(
                    mir[:, col * HP:(col + 1) * HP], sm_t[:, JDY[-dy]],
                    g1den[:, DSLOT[t]], start=(col in (0, 8)),
                    stop=(col in (7, 9)), skip_group_check=True)

            # mix groups: mirror subs (+ free-shift tap at dy=0), per dx
            def mir_sub(t):
                nc.vector.tensor_sub(
                    wstk[:, NEGSLOT[t]],
                    mir[:, MIRCOL[t] * HP:(MIRCOL[t] + 1) * HP], lnn_in1(t))

            mirv = mir[:, 0:640].rearrange("p (s h) -> p s h", s=10)
            # dx=1: ring-2 outer pair (slots 33,37; mir cols 2,5) and ring-1
            # inner pair (slots 34,36; mir cols 3,4), both uniform strides
            wstk4 = wstk[:].rearrange("p s h -> p (s h)")
            nc.vector.tensor_sub(
                wstk4[:, 33 * HP:38 * HP].rearrange(
                    "p (s h) -> p s h", s=5)[:, 0:5:4],
                mirv[:, 2:6:3], lnn2[:, 9:11])
            nc.vector.tensor_sub(
                wstk[:, 34:37:2], mirv[:, 3:5],
                lnn[:, None, 3:67].broadcast_to((W, 2, HP)))
            nc.vector.tensor_sub(wstk[:, NEGSLOT[(1, 0)]],
                                 g1x2[:, DSLOT[(1, 0)], 2:66],
                                 lnn_in1((1, 0)))
            gproduct("n", 33, [(1, dy) for dy in (2, 1, 0, -1, -2)])
            # dx=2: all mirrors ring-2, two contiguous pairs
            nc.vector.tensor_sub(wstk[:, 38:40], mirv[:, 6:8],
                                 lnn2[:, 11:13])
            nc.vector.tensor_sub(wstk[:, 41:43], mirv[:, 8:10],
                                 lnn2[:, 14:16])
            nc.vector.tensor_sub(wstk[:, NEGSLOT[(2, 0)]],
                                 g1x2[:, DSLOT[(2, 0)], 2:66],
                                 lnn_in1((2, 0)))
            gproduct("n", 38, [(2, dy) for dy in (2, 1, 0)])
            for t in ((0, 2), (0, 1)):
                mir_sub(t)
            # finish with singleton products: the PE's end-of-stream matmul
            # backlog after the last DVE op shrinks to one tap (3 matmuls)
            gproduct("n", 31, [(0, 2)])
            gproduct("n", 41, [(2, -1), (2, -2)])
            gproduct("n", 32, [(0, 1)])

            # ---- evac + stores ----
            on_t = io.tile([W, C, HP], f32, tag="on")
            op_t = io.tile([W, C, HP], f32, tag="op")
            nc.scalar.activation(op_t[:].rearrange("p c h -> p (c h)"),
                                 accp[:, 0:C * HP], Act.Copy,
                                 bias=0.0, scale=-5.0)
            nc.gpsimd.dma_start(out=o_d[1], in_=op_t[:])
            onf = on_t[:].rearrange("p c h -> p (c h)")
            nc.scalar.activation(onf[:, 0:672], accn[:, 0:672], Act.Copy,
                                 bias=0.0, scale=5.0)
            nc.vector.tensor_scalar_mul(onf[:, 672:C * HP],
                                        accn[:, 672:C * HP], 5.0)
            nc.sync.dma_start(out=o_d[0][:HW2], in_=on_t[:HW2])
            nc.scalar.dma_start(out=o_d[0][HW2:], in_=on_t[HW2:])
    nc.finalize()
    return nc


def _host_inputs(input, feats, clsbd_feats):
    x = np.asarray(input, np.float32)
    f = np.asarray(feats, np.float32)
    s = np.asarray(clsbd_feats, np.float32)[:, 0]

    xpad = np.zeros((B, C, H + 4, W), np.float32)
    xpad[:, :, 2:2 + H] = x
    fpad = np.full((B, D, H + 8, W), BIGPAD, np.float32)
    fpad[:, :, 4:4 + H] = f
    spad = np.zeros((B, H + 6, W), np.float32)
    spad[:, 3:3 + H] = s

    def shift_w(arr, dy, fill):
        out = np.full_like(arr, fill)
        if dy >= 0:
            out[..., :W - dy] = arr[..., dy:]
        else:
            out[..., -dy:] = arr[..., :W + dy]
        return out

    sm = np.zeros((W, 6, W), np.float16)
    for j, dy in enumerate(DYS5):
        a, bnd = max(0, -dy), W - max(0, dy)
        for mcol in range(a, bnd):
            sm[mcol + dy, j, mcol] = 1.0
    sm[:, 5][np.arange(W), np.arange(W)] = np.float16(C0N)

    maps = []
    for i in range(8):
        b, half = i // 2, i % 2
        h0 = half * HP
        fsw = np.stack([shift_w(fpad[b, :, h0:h0 + FE], dy, BIGPAD)
                        for dy in DYS5])          # [5, D, FE, W]
        fse = fsw.transpose(3, 0, 1, 2).astype(np.float16)
        fso = np.full_like(fse, BIGPAD)
        fso[..., :FE - 1] = fse[..., 1:]
        fsh = np.ascontiguousarray(np.stack([fse, fso], axis=1))
        ssw = np.stack([shift_w(spad[b, h0:h0 + SE], dy, 0.0)
                        for dy in (-1, 0, 1)])    # [3, SE, W]
        xw = xpad[b, :, h0:h0 + HE].transpose(2, 0, 1).reshape(W, -1)
        sw = ssw.transpose(2, 0, 1).reshape(W, -1)
        maps.append({
            "xss": np.ascontiguousarray(np.concatenate(
                [xw.astype(np.float16),
                 np.ascontiguousarray(
                     sw.astype(np.float32)).view(np.float16)], axis=1)),
            "fs": fsh,
            "sm": sm,
        })
    return maps


_last_results = None


def kernel(input, feats, clsbd_feats, label=None, **_ignored):
    global _last_results
    from concourse.bass_utils import run_bass_kernel_spmd

    in_maps = _host_inputs(input, feats, clsbd_feats)
    if "nc" not in _cache:
        _cache["nc"] = _build()
    res = run_bass_kernel_spmd(_cache["nc"], in_maps, list(range(8)))
    _last_results = res

    out = np.empty((2, B, C, H, W), np.float32)
    for i in range(8):
        b, half = i // 2, i % 2
        h0 = half * HP
        out[:, b, :, h0:h0 + HP] = res.results[i]["out"].transpose(0, 2, 3, 1)
    return out


# revision 50
# speedup vs baseline: 1.0043x; 1.0043x over previous
"""ClsbdCRF message-passing kernel for 8 Trainium2 NeuronCores.

Sharding: core i handles batch b = i//2 and image-row half i%2 (64 output
rows each, halos sliced host-side).  Per-core SBUF layout puts W=128 on
partitions and (C, H) on the free dimension.

Formulation ("input frame"): for every tap t the reference computes
  msg[c,p] += w_t[p] * xp[c, p+t].
We instead build the tap weight shifted into the *input* frame,
  w'_t[u] = w_t[u - t],
multiply v_t = w'_t * xp (per-tap elementwise product, fp16, DVE 2x mode),
and let the PE do the shift-and-accumulate into PSUM with 0/1 shift-matrix
stationaries:  msg[w] += v_t[w + dy_t]  (dx_t handled as a free-dim offset
baked into the exact 64-wide window of each product tile).

Products for taps sharing dx share the same xp window, so they are fused
into one stacked [W, K, C, 64] DVE multiply (weight slots grouped by dx);
the PE accumulation matmuls read per-tap slices of the group tile.

Weight structure exploited:
  * pairwise gaussian symmetry: g1'_t = g1_{-t}; 12 direct tiles cover 12
    taps for free, the other 10 (dy!=0) are small PE partition shifts.
  * clsbd ring-1: g2'_t[u] = s[u] for all 8 taps -> the pos-stream ring-1
    products depend only on dx -> 3 shared products, 8 matmuls.
  * ring-2: g2'_t[u] = max(s[u], s[u+d1], s[u+d2]) with |d|<=1.
  * center tap: msg_neg += c0 * xp via a scaled-identity stationary.
All products/weights fp16 (rel err ~5e-4, tolerance 2e-2); PSUM fp32.
"""

import math

import numpy as np

B, C, H, W, D = 4, 21, 128, 128, 5
EPS = 1e-5
HP = 64          # output rows per core
HE = HP + 4      # x row extent (halo 2)        frame: q in [0,68), idx = q
FE = HP + 8      # feats row extent (halo 4)    idx = q + 2
SE = HP + 6      # clsbd row extent (halo 3)    idx = q + 1
G1W = 66         # direct-gaussian window: q in [0,66)
BIGPAD = 1000.0
C0N = 2.0 - math.log(EPS)   # center neg weight (final *5 at evac)

RING1 = [(-1, -1), (-1, 0), (-1, 1), (0, -1), (0, 1), (1, -1), (1, 0), (1, 1)]
RING2 = [(-2, -2), (-2, -1), (-2, 0), (-2, 1), (-2, 2), (-1, -2), (-1, 2),
         (0, -2), (0, 2), (1, -2), (1, 2), (2, -2), (2, -1), (2, 0), (2, 1),
         (2, 2)]
EXP1 = [0, 0, 1, 2, 2, 0, 2, 3, 4, 5, 7, 5, 5, 6, 7, 7]
EXP2 = [0, 1, 1, 1, 2, 3, 4, 3, 4, 3, 4, 5, 6, 6, 6, 7]
ALLT = [(dx, dy) for dx in range(-2, 3) for dy in range(-2, 3)
        if (dx, dy) != (0, 0)]
DIRTAPS = [t for t in ALLT if t > (0, 0)]           # direct-gaussian taps
DYS5 = [-2, -1, 0, 1, 2]
JDY = {dy: j for j, dy in enumerate(DYS5)}           # fs / sm slot per dy

# direct-gaussian stack, dx-major (dy ascending inside) so the batched neg
# weight subs read uniform stride-1 slot runs
DSTACK = sorted(DIRTAPS, key=lambda t: (t[0], t[1]))
DSLOT = {t: i for i, t in enumerate(DSTACK)}

# canonical ring-2 order: dx-major, dy DESCENDING (matches neg slot order)
R2ORD = [t for dx in DYS5
         for t in sorted((u for u in RING2 if u[0] == dx),
                         key=lambda u: -u[1])]
R2IDX = {t: i for i, t in enumerate(R2ORD)}

# weight-stack slots (fp16, exact 64-wide windows)
PR1SLOT = {dx: dx + 1 for dx in (-1, 0, 1)}          # 0..2 pos ring-1 (by dx)
PR2SLOT = {t: 3 + R2IDX[t] for t in RING2}           # 3..18 pos ring-2
# neg slot groups (dy descending): [dir dx=-2 x5][dir dx=-1 x5][dir dx=0 x2]
#                                  [mir dx=0 x2][mix dx=1 x5][mix dx=2 x5]
NEGORD = ([(-2, dy) for dy in (2, 1, 0, -1, -2)]
          + [(-1, dy) for dy in (2, 1, 0, -1, -2)]
          + [(0, -1), (0, -2)]
          + [(0, 2), (0, 1)]
          + [(1, dy) for dy in (2, 1, 0, -1, -2)]
          + [(2, dy) for dy in (2, 1, 0, -1, -2)])
NEGSLOT = {t: 19 + i for i, t in enumerate(NEGORD)}
NW = 43

# mirror psum column layout (order = wstk mix-group order, fs excluded)
MIRTAPS = [(0, 2), (0, 1), (1, 2), (1, 1), (1, -1), (1, -2),
           (2, 2), (2, 1), (2, -1), (2, -2)]
MIRCOL = {t: i for i, t in enumerate(MIRTAPS)}

FSL = [(0, 512), (512, 1024), (1024, C * HP)]        # psum bank chunks
NMM = {"n": 25, "p": 24}                             # matmuls per chunk

_cache = {}


def _build():
    import concourse.bacc as bacc
    import concourse.mybir as mybir
    from concourse.tile import TileContext

    f32 = mybir.dt.float32
    f16 = mybir.dt.float16
    Act = mybir.ActivationFunctionType
    Alu = mybir.AluOpType

    nc = bacc.Bacc()
    xs_d = nc.declare_dram_parameter("xss", [W, C * HE + 6 * SE], f16,
                                     isOutput=False)
    f_d = nc.declare_dram_parameter("fs", [W, 2, 5, D, FE], f16,
                                    isOutput=False)
    m_d = nc.declare_dram_parameter("sm", [W, 6, W], f16, isOutput=False)
    o_d = nc.declare_dram_parameter("out", [2, W, C, HP], f32, isOutput=True)

    with TileContext(nc) as tc:
        with (
            tc.tile_pool(name="io", bufs=1) as io,
            tc.tile_pool(name="wk", bufs=1) as wk,
            tc.tile_pool(name="sc", bufs=1) as scp,
            tc.tile_pool(name="vp", bufs=3) as vp,
            tc.tile_pool(name="ps", bufs=1, space="PSUM") as psp,
        ):
            # ---- loads (all plain contiguous DMAs, spread over queues) ----
            xss_t = io.tile([W, C * HE + 6 * SE], f16, tag="xss")
            fs_t = io.tile([W, 2, 5, D, FE], f16, tag="fs")
            sm_t = io.tile([W, 6, W], f16, tag="sm")
            bc_t = io.tile([W, 3], f32, tag="bc")
            # biases via memsets: a [W,3] DMA costs a full 128-descriptor
            # chain (~6us) and would gate lnx
            nc.gpsimd.memset(bc_t[:, 0:1], EPS)
            nc.gpsimd.memset(bc_t[:, 1:2], math.log(2.0))
            nc.gpsimd.memset(bc_t[:, 2:3], 1.0 + EPS)
            # prime the Ln activation-table set during the DMA wait so lnx
            # does not pay the ~1.3us ACT_TABLE_LOAD on the critical chain
            warm = wk.tile([W, 1], f32, tag="warm")
            with tc.high_priority():
                nc.scalar.activation(warm[:], bc_t[:, 0:1], Act.Ln,
                                     bias=bc_t[:, 0:1], scale=1.0)
            HW2 = W // 2
            nc.sync.dma_start(out=xss_t[:HW2], in_=xs_d[:HW2])
            nc.scalar.dma_start(out=xss_t[HW2:], in_=xs_d[HW2:])
            nc.gpsimd.dma_start(out=fs_t[:], in_=f_d[:])
            nc.sync.dma_start(out=sm_t[:], in_=m_d[:])
            x_t = xss_t[:, 0:C * HE].rearrange("p (c h) -> p c h", c=C)
            ss_t = xss_t[:, C * HE:].bitcast(f32).rearrange(
                "p (s h) -> p s h", s=3)
            b_eps = bc_t[:, 0:1]
            b_ln2 = bc_t[:, 1:2]
            b_1eps = bc_t[:, 2:3]

            accn = psp.tile([W, 1536], f32, tag="accn")
            accp = psp.tile([W, 1536], f32, tag="accp")
            mir = psp.tile([W, 1024], f32, tag="mir")

            s0 = ss_t[:, 1]                       # s[u] at idx u+1

            # ---- ring-2 clsbd gaussians (R2ORD slots, runs batched) ----
            g2r2 = wk.tile([W, 16, HP], f32, tag="g2r2")
            deltas = []
            for t in R2ORD:
                dx, dy = t
                k = RING2.index(t)
                par = sorted({EXP1[k], EXP2[k]})
                deltas.append((2 + dx,
                               [(RING1[e][0] - dx, RING1[e][1] - dy)
                                for e in par]))
            sl = 0
            while sl < 16:          # pass 1: runs sharing (u0, delta1)
                u0, ds = deltas[sl]
                n = 1
                while sl + n < 16 and deltas[sl + n][0] == u0 \
                        and deltas[sl + n][1][0] == ds[0]:
                    n += 1
                dxx, dyy = ds[0]
                nc.vector.tensor_max(
                    g2r2[:, sl:sl + n],
                    s0[:, None, u0 + 1:u0 + 65].broadcast_to((W, n, HP)),
                    ss_t[:, 1 + dyy, None,
                         u0 + dxx + 1:u0 + dxx + 65].broadcast_to(
                             (W, n, HP)))
                sl += n
            for sl, (u0, ds) in enumerate(deltas):   # pass 2: 2nd parent
                if len(ds) > 1:
                    dxx, dyy = ds[1]
                    nc.vector.tensor_max(
                        g2r2[:, sl], g2r2[:, sl],
                        ss_t[:, 1 + dyy, u0 + dxx + 1:u0 + dxx + 65])

            # ---- polarness -> xp (fp16); high priority so the scheduler
            # prefers this chain over the fs-gated g1 subs (fills the DVE
            # bubble while the fs DMA is still landing) ----
            with tc.high_priority():
                lnx = scp.tile([W, C, HE], f16, tag="lnx")
                nc.scalar.activation(lnx[:], x_t, Act.Ln, bias=b_eps,
                                     scale=1.0)
                xl = scp.tile([W, C, HE], f16, tag="xl")
                nc.vector.tensor_mul(xl[:], x_t, lnx[:])
                e10 = scp.tile([W, 10, HE], f16, tag="e10")
                nc.vector.tensor_add(e10[:], xl[:, 0:10], xl[:, 10:20])
                e5 = scp.tile([W, 5, HE], f16, tag="e5")
                nc.vector.tensor_add(e5[:], e10[:, 0:5], e10[:, 5:10])
                e2 = scp.tile([W, 2, HE], f16, tag="e2")
                nc.vector.tensor_add(e2[:], e5[:, 0:2], e5[:, 2:4])
                ent = wk.tile([W, HE], f16, tag="ent")
                nc.vector.tensor_add(ent[:], e2[:, 0], e2[:, 1])
                nc.vector.tensor_add(ent[:], ent[:], e5[:, 4])
                nc.vector.tensor_add(ent[:], ent[:], xl[:, 20])
                pl = wk.tile([W, HE], f16, tag="pl")
                nc.scalar.activation(pl[:], ent[:], Act.Copy,
                                     bias=1.0, scale=1.0 / math.log(C))
                xp16e = io.tile([W, C, HE], f16, tag="xp16e")
                nc.vector.tensor_mul(
                    xp16e[:], x_t,
                    pl[:, None, :].broadcast_to((W, C, HE)))
                xp16o = io.tile([W, C, G1W], f16, tag="xp16o")
                nc.vector.tensor_copy(xp16o[:], xp16e[:, :, 1:1 + G1W])
            xpc = io.tile([W, C, HP], f16, tag="xpc")
            nc.vector.tensor_copy(xpc[:], xp16e[:, :, 2:2 + HP])

            def xp_win(dx):
                o = 2 + dx
                if o % 2 == 0:
                    return xp16e[:, :, o:o + HP]
                return xp16o[:, :, o - 1:o - 1 + HP]

            # ---- early scalar weights from s only ----
            wstk = wk.tile([W, NW, HP], f16, tag="wstk")
            lnn = wk.tile([W, HE], f16, tag="lnn")
            nc.scalar.activation(lnn[:], s0[:, 1:1 + HE], Act.Ln,
                                 bias=b_eps, scale=1.0)
            for dx in (-1, 0, 1):
                u0 = 2 + dx
                nc.scalar.activation(wstk[:, PR1SLOT[dx]],
                                     s0[:, u0 + 1:u0 + 65], Act.Ln,
                                     bias=b_1eps, scale=-1.0)

            # ---- matmul emission helpers (psum accumulation groups) ----
            cnt = {("n", i): 0 for i in range(3)} | {("p", i): 0
                                                     for i in range(3)}

            def emit_mm(stream, vflat, dy):
                ps = accn if stream == "n" else accp
                for ci, (n0, n1) in enumerate(FSL):
                    cnt[(stream, ci)] += 1
                    c = cnt[(stream, ci)]
                    nc.tensor.matmul(ps[:, n0:n1], sm_t[:, JDY[dy]],
                                     vflat[:, n0:n1], start=(c == 1),
                                     stop=(c == NMM[stream]),
                                     skip_group_check=True)

            def product(stream, wslot, dx, dys):
                # single-tap product, possibly accumulated with several dys
                v = vp.tile([W, C, HP], f16, tag="v")
                nc.vector.tensor_mul(
                    v[:], wstk[:, wslot, None, :].broadcast_to((W, C, HP)),
                    xp_win(dx))
                vflat = v[:].rearrange("p c h -> p (c h)")
                for dy in dys:
                    emit_mm(stream, vflat, dy)

            def gproduct(stream, slot0, taps):
                # fused product for K same-dx taps: [W, K, C, HP]
                dx = taps[0][0]
                K = len(taps)
                vg = vp.tile([W, K, C, HP], f16, tag=f"vg{K}")
                nc.vector.tensor_mul(
                    vg[:],
                    wstk[:, slot0:slot0 + K, None, :].broadcast_to(
                        (W, K, C, HP)),
                    xp_win(dx)[:, None, :, :].broadcast_to((W, K, C, HP)))
                for k, (tdx, tdy) in enumerate(taps):
                    emit_mm(stream, vg[:, k].rearrange("p c h -> p (c h)"),
                            tdy)

            # center tap first: warms PE early, opens the accn groups
            for ci, (n0, n1) in enumerate(FSL):
                cnt[("n", ci)] += 1
                nc.tensor.matmul(accn[:, n0:n1], sm_t[:, 5],
                                 xpc[:].rearrange("p c h -> p (c h)")[:, n0:n1],
                                 start=True, stop=(cnt[("n", ci)] == NMM["n"]),
                                 skip_group_check=True)

            # ---- pairwise gaussian chain ----
            dstk = scp.tile([W, 12, D, G1W], f16, tag="dstk")
            for t in DIRTAPS:
                mdx, mdy = t
                o = 2 + mdx
                if o % 2 == 0:
                    in1 = fs_t[:, 0, JDY[mdy], :, o:o + G1W]
                else:
                    in1 = fs_t[:, 1, JDY[mdy], :, o - 1:o - 1 + G1W]
                nc.vector.tensor_sub(
                    dstk[:, DSLOT[t]], fs_t[:, 0, 2, :, 2:2 + G1W], in1)
            sq = scp.tile([W, 12, D, G1W], f16, tag="sq")
            for a, b2 in ((0, 4), (4, 8), (8, 12)):
                nc.scalar.activation(sq[:, a:b2], dstk[:, a:b2],
                                     Act.Square, bias=0.0, scale=1.0)

            # pos ring-1: 3 shared products (weight depends only on dx),
            # each accumulated with 2-3 different dy shift matrices
            for dx in (-1, 0, 1):
                dys = [dy for dy in (-1, 0, 1) if (dx, dy) != (0, 0)]
                product("p", PR1SLOT[dx], dx, dys)

            # ---- sum over D (fp16 adds, 2x mode) ----
            q01 = scp.tile([W, 12, G1W], f16, tag="q01")
            nc.vector.tensor_add(q01[:], sq[:, :, 0], sq[:, :, 1])
            q23 = scp.tile([W, 12, G1W], f16, tag="q23")
            nc.vector.tensor_add(q23[:], sq[:, :, 2], sq[:, :, 3])
            nc.vector.tensor_add(q01[:], q01[:], q23[:])
            ssum = scp.tile([W, 12, G1W], f16, tag="ssum")
            nc.vector.tensor_add(ssum[:], q01[:], sq[:, :, 4])

            # ---- ring-2 weights (scalar; only need g2r2) + exp ----
            lnn2 = wk.tile([W, 16, HP], f16, tag="lnn2")
            nc.scalar.activation(lnn2[:], g2r2[:], Act.Ln,
                                 bias=b_eps, scale=1.0)
            nc.scalar.activation(wstk[:, 3:19], g2r2[:], Act.Ln,
                                 bias=b_1eps, scale=-1.0)
            g1x2 = wk.tile([W, 12, G1W], f16, tag="g1x2")
            nc.scalar.activation(g1x2[:], ssum[:], Act.Exp,
                                 bias=b_ln2, scale=-0.5)
            for dx in DYS5:
                taps = [t for t in R2ORD if t[0] == dx]
                gproduct("p", PR2SLOT[taps[0]], taps)

            def lnn_in1(t):
                if t in RING2:
                    return lnn2[:, R2IDX[t]]
                return lnn[:, 2 + t[0]:2 + t[0] + HP]

            # ---- neg direct weights: batched subs with uniform strides ----
            # dx=-2 group: all ring-2; direct slots (2,-dy) = 7..11 asc
            nc.vector.tensor_sub(wstk[:, 19:24], g1x2[:, 7:12, 0:64],
                                 lnn2[:, 0:5])
            gproduct("n", 19, [(-2, dy) for dy in (2, 1, 0, -1, -2)])
            # dx=-1 group: ring-1 middle run + 2 ring-2 singletons
            nc.vector.tensor_sub(
                wstk[:, 25:28], g1x2[:, 3:6, 1:65],
                lnn[:, None, 1:65].broadcast_to((W, 3, HP)))
            nc.vector.tensor_sub(wstk[:, 24], g1x2[:, 2, 1:65],
                                 lnn2[:, R2IDX[(-1, 2)]])
            nc.vector.tensor_sub(wstk[:, 28], g1x2[:, 6, 1:65],
                                 lnn2[:, R2IDX[(-1, -2)]])
            gproduct("n", 24, [(-1, dy) for dy in (2, 1, 0, -1, -2)])
            # dx=0 direct: (0,-1) ring1, (0,-2) ring2
            nc.vector.tensor_sub(wstk[:, 29], g1x2[:, 0, 2:66],
                                 lnn[:, 2:66])
            nc.vector.tensor_sub(wstk[:, 30], g1x2[:, 1, 2:66],
                                 lnn2[:, R2IDX[(0, -2)]])
            gproduct("n", 29, [(0, -1), (0, -2)])

            # ---- mirror taps via per-tap PE partition shifts ----
            g1den = wk.tile([W, 12, HP], f16, tag="g1den")
            nc.scalar.activation(g1den[:], g1x2[:, :, 2:2 + HP],
                                 Act.Copy, bias=0.0, scale=1.0)
            for t in MIRTAPS:
                dx, dy = t
                col = MIRCOL[t]
                nc.tensor.matmul(
                    mir[:, col * HP:(col + 1) * HP], sm_t[:, JDY[-dy]],
                    g1den[:, DSLOT[t]], start=(col in (0, 8)),
                    stop=(col in (7, 9)), skip_group_check=True)

            # mix groups: mirror subs (+ free-shift tap at dy=0), per dx
            def mir_sub(t):
                nc.vector.tensor_sub(
                    wstk[:, NEGSLOT[t]],
                    mir[:, MIRCOL[t] * HP:(MIRCOL[t] + 1) * HP], lnn_in1(t))

            mirv = mir[:, 0:640].rearrange("p (s h) -> p s h", s=10)
            # dx=1: ring-2 outer pair (slots 33,37; mir cols 2,5) and ring-1
            # inner pair (slots 34,36; mir cols 3,4), both uniform strides
            wstk4 = wstk[:].rearrange("p s h -> p (s h)")
            nc.vector.tensor_sub(
                wstk4[:, 33 * HP:38 * HP].rearrange(
                    "p (s h) -> p s h", s=5)[:, 0:5:4],
                mirv[:, 2:6:3], lnn2[:, 9:11])
            nc.vector.tensor_sub(
                wstk[:, 34:37:2], mirv[:, 3:5],
                lnn[:, None, 3:67].broadcast_to((W, 2, HP)))
            nc.vector.tensor_sub(wstk[:, NEGSLOT[(1, 0)]],
                                 g1x2[:, DSLOT[(1, 0)], 2:66],
                                 lnn_in1((1, 0)))
            gproduct("n", 33, [(1, dy) for dy in (2, 1, 0, -1, -2)])
            # dx=2: all mirrors ring-2, two contiguous pairs
            nc.vector.tensor_sub(wstk[:, 38:40], mirv[:, 6:8],
                                 lnn2[:, 11:13])
            nc.vector.tensor_sub(wstk[:, 41:43], mirv[:, 8:10],
                                 lnn2[:, 14:16])
            nc.vector.tensor_sub(wstk[:, NEGSLOT[(2, 0)]],
                                 g1x2[:, DSLOT[(2, 0)], 2:66],
                                 lnn_in1((2, 0)))
            gproduct("n", 38, [(2, dy) for dy in (2, 1, 0)])
            gproduct("n", 41, [(2, -1), (2, -2)])
            for t in ((0, 2), (0, 1)):
                mir_sub(t)
            gproduct("n", 31, [(0, 2), (0, 1)])

            # ---- evac + stores ----
            on_t = io.tile([W, C, HP], f32, tag="on")
            op_t = io.tile([W, C, HP], f32, tag="op")
            nc.scalar.activation(op_t[:].rearrange("p c h -> p (c h)"),
                                 accp[:, 0:C * HP], Act.Copy,
                                 bias=0.0, scale=-5.0)
            nc.gpsimd.dma_start(out=o_d[1], in_=op_t[:])
            onf = on_t[:].rearrange("p c h -> p (c h)")
            nc.scalar.activation(onf[:, 0:672], accn[:, 0:672], Act.Copy,
                                 bias=0.0, scale=5.0)
            nc.vector.tensor_scalar_mul(onf[:, 672:C * HP],
                                        accn[:, 672:C * HP], 5.0)
            nc.sync.dma_start(out=o_d[0][:HW2], in_=on_t[:HW2])
            nc.scalar.dma_start(out=o_d[0][HW2:], in_=on_t[HW2:])
    nc.finalize()
    return nc


def _host_inputs(input, feats, clsbd_feats):
    x = np.asarray(input, np.float32)
    f = np.asarray(feats, np.float32)
    s = np.asarray(clsbd_feats, np.float32)[:, 0]

    xpad = np.zeros((B, C, H + 4, W), np.float32)
    xpad[:, :, 2:2 + H] = x
    fpad = np.full((B, D, H + 8, W), BIGPAD, np.float32)
    fpad[:, :, 4:4 + H] = f
    spad = np.zeros((B, H + 6, W), np.float32)
    spad[:, 3:3 + H] = s

    def shift_w(arr, dy, fill):
        out = np.full_like(arr, fill)
        if dy >= 0:
            out[..., :W - dy] = arr[..., dy:]
        else:
            out[..., -dy:] = arr[..., :W + dy]
        return out

    sm = np.zeros((W, 6, W), np.float16)
    for j, dy in enumerate(DYS5):
        a, bnd = max(0, -dy), W - max(0, dy)
        for mcol in range(a, bnd):
            sm[mcol + dy, j, mcol] = 1.0
    sm[:, 5][np.arange(W), np.arange(W)] = np.float16(C0N)

    maps = []
    for i in range(8):
        b, half = i // 2, i % 2
        h0 = half * HP
        fsw = np.stack([shift_w(fpad[b, :, h0:h0 + FE], dy, BIGPAD)
                        for dy in DYS5])          # [5, D, FE, W]
        fse = fsw.transpose(3, 0, 1, 2).astype(np.float16)
        fso = np.full_like(fse, BIGPAD)
        fso[..., :FE - 1] = fse[..., 1:]
        fsh = np.ascontiguousarray(np.stack([fse, fso], axis=1))
        ssw = np.stack([shift_w(spad[b, h0:h0 + SE], dy, 0.0)
                        for dy in (-1, 0, 1)])    # [3, SE, W]
        xw = xpad[b, :, h0:h0 + HE].transpose(2, 0, 1).reshape(W, -1)
        sw = ssw.transpose(2, 0, 1).reshape(W, -1)
        maps.append({
            "xss": np.ascontiguousarray(np.concatenate(
                [xw.astype(np.float16),
                 np.ascontiguousarray(
                     sw.astype(np.float32)).view(np.float16)], axis=1)),
            "fs": fsh,
            "sm": sm,
        })
    return maps


_last_results = None


def kernel(input, feats, clsbd_feats, label=None, **_ignored):
    global _last_results
    from concourse.bass_utils import run_bass_kernel_spmd

    in_maps = _host_inputs(input, feats, clsbd_feats)
    if "nc" not in _cache:
        _cache["nc"] = _build()
    res = run_bass_kernel_spmd(_cache["nc"], in_maps, list(range(8)))
    _last_results = res

    out = np.empty((2, B, C, H, W), np.float32)
    for i in range(8):
        b, half = i // 2, i % 2
        h0 = half * HP
        out[:, b, :, h0:h0 + HP] = res.results[i]["out"].transpose(0, 2, 3, 1)
    return out
